# revision 12
# baseline (speedup 1.0000x reference)
"""LIF (leaky integrate-and-fire) scan kernel for Trainium2, 8 NeuronCores.

Reference semantics (fp32, T=8 innermost axis):
    mem = 0
    for t in range(T):
        mem = mem * 0.5 + x[..., t]
        s[..., t] = (mem >= 1.0)
        mem = mem * (1.0 - s[..., t])

Sharding: data-parallel over the leading dim (64 -> 8 per core).

Memory-roofline design: the input must stream 32 MiB/core of fp32, but the
output is binary, so it leaves the device as uint8 (8 MiB/core instead of
32 MiB).  The host maps spikes back with (y == 1) -> f32, immune to
whether the device's f32->u8 conversion saturates or wraps.

Per-core layout is chunk-major/t-minor: x[p, (c*T + t)*CH + n].  The
per-timestep ops are placed across FOUR engines by measured cost
(DVE stt 1.19us, DVE ts-2x 0.645us, Act 1.15us, Pool tt 3.18us, PE
identity-matmul A 3.2us per [128,1024] strip):

    A (m = 0.5 r + x):  DVE scalar_tensor_tensor for most steps; ~18
        steps/group-set run on the PE as two accumulating identity
        matmuls per 512-wide half (0.5I @ r then I @ x) into PSUM --
        exact, since each output is a single product.
    B (y_t = sign(m-1) -> u8): Act engine, reads SBUF or PSUM directly.
    C (r = m * [m < 1]): three flavours --
        stt:  one DVE scalar_tensor_tensor (SBUF m);
        pool: DVE tensor_scalar 2x gate + Pool tensor_tensor mult
              (Pool rejects stt and cannot read PSUM);
        PE-m: Act copies PSUM m to SBUF, then the pool flavour.

Engine-balance (measured): DVE ~95us, Act ~95us, Pool ~95us, PE ~57us,
all under/near the DMA floor.  Input strips split qSP (t=0..4) / qAct
(t=5..7) hardware DGE queues, issued in per-group prologues so ring-slot
waits never deadlock an engine sequencer; outputs ride qPool.
"""

import numpy as np

import concourse.bass as bass
import concourse.tile as tile
from concourse import bacc, mybir
from concourse.bass_utils import run_bass_kernel_spmd

P = 128           # SBUF partitions
T = 8             # timesteps (innermost axis of the original input)
NPB = 8192        # neurons per partition per core: 8*128*32*32 / 128
CH = 1024         # neurons per chunk (per partition)
NCH = NPB // CH   # 8 chunks
HMM = 512         # matmul moving-operand half width
GROUPS = [[0, 1, 2], [3, 4, 5], [6, 7]]

THRESH = 1.0
DECAY = 0.5
F32 = mybir.dt.float32
U8 = mybir.dt.uint8
N_CORES = 8

Alu = mybir.AluOpType
Act = mybir.ActivationFunctionType

QACT_T = 5        # strips with t >= QACT_T ride the qAct queue


C_POOL_CHUNKS = (1, 2, 4, 5, 7)   # chunks whose reset runs gate+Pool-mult


def _schedule():
    """Returns (a_pe, c_pool): sets of (c, t) steps.

    a_pe empty: v3 measured that putting the update A on the PE left every
    engine ~50% stalled on cross-engine latency (283us vs the 100us
    busy-time prediction) -- the PSUM->SBUF copy sits on the recurrence's
    critical path.  The chain stays on the DVE with chunk-interleaved
    in-order issue; B hangs off-chain on Act.

    c_pool: for these chunks the reset becomes DVE 2x-gate + Pool
    tensor_tensor mult, trading 1.21us of DVE for 0.65us DVE + 2.8us Pool.
    At most 2 Pool resets land per round, so the Pool result returns well
    before the next round's A consumes it.
    """
    c_pool = {
        (c, t)
        for chunks in GROUPS
        for c in chunks
        if c in C_POOL_CHUNKS
        for t in range(T - 1)
    }
    return set(), c_pool


def _build() -> bass.Bass:
    nc = bacc.Bacc("TRN2", target_bir_lowering=False, debug=False)
    x = nc.dram_tensor("x", [P, NCH * T * CH], F32, kind="ExternalInput").ap()
    w = nc.dram_tensor("w", [P, 256], F32, kind="ExternalInput").ap()
    y = nc.dram_tensor("y", [P, NCH * T * CH], U8, kind="ExternalOutput").ap()

    a_pe, c_pool = _schedule()

    with tile.TileContext(nc) as tc:
        with (
            tc.tile_pool(name="consts", bufs=1) as cpool,
            tc.tile_pool(name="xs", bufs=26) as xpool,
            tc.tile_pool(name="ys", bufs=5) as ypool,
            tc.tile_pool(name="ms", bufs=4) as mpool,
            tc.tile_pool(name="rs", bufs=4) as rpool,
            tc.tile_pool(name="mcopy", bufs=3) as mcpool,
            tc.tile_pool(name="gates", bufs=3) as gpool,
            tc.tile_pool(name="psum", bufs=4, space="PSUM") as ppool,
        ):
            neg_thresh = cpool.tile([P, 1], F32, tag="negth", name="neg_thresh")
            nc.gpsimd.memset(neg_thresh[:], -float(THRESH))
            wt = cpool.tile([P, 256], F32, tag="w", name="wt")
            nc.sync.dma_start(wt[:], w[:])
            w_id = wt[:, 0:128]
            w_half = wt[:, 128:256]

            def spike(c, t, m_ap):
                yslc = yt[c][:, t * CH : (t + 1) * CH]
                nc.scalar.activation(yslc, m_ap, Act.Sign, bias=neg_thresh[:])

            def reset(c, t, src_ap):
                # r[c] = src * [src < 1]
                if (c, t) in c_pool or (c, t) in a_pe:
                    g = gpool.tile([P, CH], F32, tag="g", name=f"g{c}_{t}")
                    nc.vector.tensor_scalar(
                        g[:], src_ap, THRESH, None, Alu.is_lt, Alu.bypass
                    )
                    nc.gpsimd.tensor_tensor(r[c][:], g[:], src_ap, Alu.mult)
                else:
                    nc.vector.scalar_tensor_tensor(
                        r[c][:], src_ap, THRESH, src_ap, Alu.is_lt, Alu.mult
                    )

            yt, r = {}, {}
            for chunks in GROUPS:
                # Prologue: this group's input strips.  t<QACT_T strips on
                # the qSP HW queue (SP never blocks); t>=QACT_T on qAct,
                # whose issue point (here, before this group's B ops) the
                # Act sequencer reaches while the previous group computes.
                xs = {c: [None] * T for c in chunks}
                for t in range(T):
                    for c in chunks:
                        st = xpool.tile([P, CH], F32, tag="x", name=f"x{c}_{t}")
                        # All input on qSP: the SP sequencer runs no compute,
                        # so strips always stream ahead of DVE consumption.
                        # (Strips routed via qAct stalled the last group 17us
                        # in v4: Act only issues them after the prior group's
                        # B ops.)
                        nc.sync.dma_start(
                            st[:], x[:, (c * T + t) * CH : (c * T + t + 1) * CH]
                        )
                        xs[c][t] = st

                for c in chunks:
                    yt[c] = ypool.tile([P, T * CH], U8, tag="y", name=f"y{c}")
                    r[c] = rpool.tile([P, CH], F32, tag="r", name=f"r{c}")

                # t = 0: mem0 = 0 so m == x_0 (SBUF strip).
                for c in chunks:
                    spike(c, 0, xs[c][0][:])
                for c in chunks:
                    reset(c, 0, xs[c][0][:])

                m = {}
                for t in range(1, T):
                    # A: m = 0.5*r + x_t
                    for c in chunks:
                        if (c, t) in a_pe:
                            pm = ppool.tile([P, CH], F32, tag="pm",
                                            name=f"pm{c}_{t}")
                            for h in range(2):
                                sl = slice(h * HMM, (h + 1) * HMM)
                                nc.tensor.matmul(
                                    pm[:, sl], w_half, r[c][:, sl],
                                    start=True, stop=False,
                                )
                                nc.tensor.matmul(
                                    pm[:, sl], w_id, xs[c][t][:, sl],
                                    start=False, stop=True,
                                )
                            m[c] = pm
                        else:
                            ms = mpool.tile([P, CH], F32, tag="m",
                                            name=f"m{c}_{t}")
                            nc.vector.scalar_tensor_tensor(
                                ms[:], r[c][:], DECAY, xs[c][t][:],
                                Alu.mult, Alu.add,
                            )
                            m[c] = ms
                    # B (+ PSUM->SBUF copy for PE steps' C)
                    for c in chunks:
                        spike(c, t, m[c][:])
                        if (c, t) in a_pe and t < T - 1:
                            mc = mcpool.tile([P, CH], F32, tag="mc",
                                             name=f"mc{c}_{t}")
                            nc.scalar.copy(mc[:], m[c][:])
                            m[c] = mc
                    # C
                    if t < T - 1:
                        for c in chunks:
                            reset(c, t, m[c][:])

                # Output: one contiguous u8 chunk per c on qPool.
                for c in chunks:
                    nc.gpsimd.dma_start(
                        y[:, c * T * CH : (c + 1) * T * CH], yt[c][:]
                    )
    nc.compile()
    return nc


_NC_CACHE: bass.Bass | None = None


def _get_nc() -> bass.Bass:
    global _NC_CACHE
    if _NC_CACHE is None:
        _NC_CACHE = _build()
    return _NC_CACHE


_W = np.concatenate(
    [np.eye(128, dtype=np.float32), 0.5 * np.eye(128, dtype=np.float32)], axis=1
)


def _run(X: np.ndarray, **spmd_kwargs):
    assert X.shape == (64, 128, 32, 32, 8), X.shape
    X = np.ascontiguousarray(X, dtype=np.float32)
    per_core = 64 // N_CORES
    # [core, p, nch, ch, t] -> chunk-major t-minor [core, p, nch, t, ch]
    Xt = np.ascontiguousarray(
        X.reshape(N_CORES, P, NCH, CH, T).transpose(0, 1, 2, 4, 3)
    )
    in_maps = [
        {"x": Xt[i].reshape(P, NCH * T * CH), "w": _W} for i in range(N_CORES)
    ]
    res = run_bass_kernel_spmd(
        _get_nc(), in_maps, core_ids=list(range(N_CORES)), **spmd_kwargs
    )
    out = np.empty_like(X)
    for i, rr in enumerate(res.results):
        s = rr["y"].reshape(P, NCH, T, CH).transpose(0, 1, 3, 2)
        out[i * per_core : (i + 1) * per_core] = (
            (s == 1).astype(np.float32).reshape(per_core, 128, 32, 32, 8)
        )
    return out, res


def kernel(X: np.ndarray) -> np.ndarray:
    out, _ = _run(X)
    return out


# revision 14
# speedup vs baseline: 1.3704x; 1.3704x over previous
"""LIF (leaky integrate-and-fire) scan kernel for Trainium2, 8 NeuronCores.

Reference semantics (fp32, T=8 innermost axis):
    mem = 0
    for t in range(T):
        mem = mem * 0.5 + x[..., t]
        s[..., t] = (mem >= 1.0)
        mem = mem * (1.0 - s[..., t])

Sharding: data-parallel over the leading dim (64 -> 8 batches per core).

Memory-roofline design: the input must stream 32 MiB/core of fp32, but
the output is binary, so it leaves the device as uint8 (8 MiB/core
instead of 32 MiB).  The host maps spikes back with (y == 1) -> f32,
immune to whether the device's f32->u8 conversion saturates or wraps.

Per-core layout is chunk-major/t-minor: x[p, (c*T + t)*CH + n], so every
strip either engine touches is unit-stride.

Op placement (measured costs: DVE stt 1.21us, Act 1.15us per [128,1024]
strip): the serial recurrence A -> C -> A stays entirely on the DVE --
    A: m = (r mult 0.5) add x_t        scalar_tensor_tensor
    C: r = (m is_lt 1) mult m          scalar_tensor_tensor
with chunks interleaved t-outer/chunk-inner inside groups of 3, so the
in-order DVE stream always has another chunk's op between a chunk's C
and its next A: the chain never stalls (measured: 2us of DVE gaps over
the whole run).  Only the terminal spike op
    B: y_t = Sign(m - 1) -> uint8      activation
hangs off-chain on the Act engine.  Alternatives measured and rejected:
Pool stt is unsupported, Pool tensor_scalar runs at 16.5us, Pool
tensor_tensor (3.2us) stalls the chain when the reset is offloaded
(v5b: +53us), and PE identity-matmul updates need an on-chain
PSUM->SBUF copy (v3: every engine ~50% latency-stalled).

DMA: input strips ride the qSP hardware DGE queue only -- the SP
sequencer runs no compute, so strips always stream ahead of the DVE
(101us supply vs 136us demand); routing any strips via qAct stalled the
last group by 17us (Act issues them only after the prior group's B ops).
Spike strips are written back per-timestep on the software qPool queue
the moment each B completes, so the drain tail after the last spike is a
single 1 KiB/partition transfer.
"""

import numpy as np

import concourse.bass as bass
import concourse.tile as tile
from concourse import bacc, mybir
from concourse.bass_utils import run_bass_kernel_spmd

P = 128           # SBUF partitions
T = 8             # timesteps (innermost axis of the original input)
NPB = 8192        # neurons per partition per core: 8*128*32*32 / 128
CH = 1024         # neurons per chunk (per partition)
NCH = NPB // CH   # 8 chunks
GROUPS = [[0, 1, 2], [3, 4, 5], [6, 7]]

THRESH = 1.0
DECAY = 0.5
F32 = mybir.dt.float32
U8 = mybir.dt.uint8
N_CORES = 8

Alu = mybir.AluOpType
Act = mybir.ActivationFunctionType


def _build() -> bass.Bass:
    nc = bacc.Bacc("TRN2", target_bir_lowering=False, debug=False)
    x = nc.dram_tensor("x", [P, NCH * T * CH], F32, kind="ExternalInput").ap()
    y = nc.dram_tensor("y", [P, NCH * T * CH], U8, kind="ExternalOutput").ap()

    with tile.TileContext(nc) as tc:
        with (
            tc.tile_pool(name="consts", bufs=1) as cpool,
            tc.tile_pool(name="xs", bufs=32) as xpool,
            tc.tile_pool(name="ys", bufs=8) as ypool,
            tc.tile_pool(name="ms", bufs=4) as mpool,
            tc.tile_pool(name="rs", bufs=4) as rpool,
        ):
            neg_thresh = cpool.tile([P, 1], F32, tag="negth", name="neg_thresh")
            nc.gpsimd.memset(neg_thresh[:], -float(THRESH))

            def spike(c, t, m_ap):
                # B: u8 spike strip, written straight out on qPool.
                ys = ypool.tile([P, CH], U8, tag="y", name=f"y{c}_{t}")
                nc.scalar.activation(ys[:], m_ap, Act.Sign, bias=neg_thresh[:])
                nc.gpsimd.dma_start(
                    y[:, (c * T + t) * CH : (c * T + t + 1) * CH], ys[:]
                )

            for chunks in GROUPS:
                xs = {c: [None] * T for c in chunks}
                for t in range(T):
                    for c in chunks:
                        st = xpool.tile([P, CH], F32, tag="x", name=f"x{c}_{t}")
                        nc.sync.dma_start(
                            st[:], x[:, (c * T + t) * CH : (c * T + t + 1) * CH]
                        )
                        xs[c][t] = st

                r = {}
                for c in chunks:
                    r[c] = rpool.tile([P, CH], F32, tag="r", name=f"r{c}")

                # t = 0: mem0 = 0 so m == x_0 (read the strip directly).
                for c in chunks:
                    spike(c, 0, xs[c][0][:])
                for c in chunks:
                    nc.vector.scalar_tensor_tensor(
                        r[c][:], xs[c][0][:], THRESH, xs[c][0][:],
                        Alu.is_lt, Alu.mult,
                    )

                for t in range(1, T):
                    m = {}
                    for c in chunks:
                        ms = mpool.tile([P, CH], F32, tag="m", name=f"m{c}_{t}")
                        nc.vector.scalar_tensor_tensor(
                            ms[:], r[c][:], DECAY, xs[c][t][:],
                            Alu.mult, Alu.add,
                        )
                        m[c] = ms
                    for c in chunks:
                        spike(c, t, m[c][:])
                    if t < T - 1:
                        for c in chunks:
                            nc.vector.scalar_tensor_tensor(
                                r[c][:], m[c][:], THRESH, m[c][:],
                                Alu.is_lt, Alu.mult,
                            )
    nc.compile()
    return nc


_NC_CACHE: bass.Bass | None = None


def _get_nc() -> bass.Bass:
    global _NC_CACHE
    if _NC_CACHE is None:
        _NC_CACHE = _build()
    return _NC_CACHE


def _run(X: np.ndarray, **spmd_kwargs):
    assert X.shape == (64, 128, 32, 32, 8), X.shape
    X = np.ascontiguousarray(X, dtype=np.float32)
    per_core = 64 // N_CORES
    # [core, p, nch, ch, t] -> chunk-major t-minor [core, p, nch, t, ch]
    Xt = np.ascontiguousarray(
        X.reshape(N_CORES, P, NCH, CH, T).transpose(0, 1, 2, 4, 3)
    )
    in_maps = [{"x": Xt[i].reshape(P, NCH * T * CH)} for i in range(N_CORES)]
    res = run_bass_kernel_spmd(
        _get_nc(), in_maps, core_ids=list(range(N_CORES)), **spmd_kwargs
    )
    out = np.empty_like(X)
    for i, rr in enumerate(res.results):
        s = rr["y"].reshape(P, NCH, T, CH).transpose(0, 1, 3, 2)
        out[i * per_core : (i + 1) * per_core] = (
            (s == 1).astype(np.float32).reshape(per_core, 128, 32, 32, 8)
        )
    return out, res


def kernel(X: np.ndarray) -> np.ndarray:
    out, _ = _run(X)
    return out


# revision 15
# speedup vs baseline: 1.4271x; 1.0414x over previous
"""LIF (leaky integrate-and-fire) scan kernel for Trainium2, 8 NeuronCores.

Reference semantics (fp32, T=8 innermost axis):
    mem = 0
    for t in range(T):
        mem = mem * 0.5 + x[..., t]
        s[..., t] = (mem >= 1.0)
        mem = mem * (1.0 - s[..., t])

Sharding: data-parallel over the leading dim (64 -> 8 batches per core).

Memory-roofline design: the input must stream 32 MiB/core of fp32, but
the output is binary, so it leaves the device as uint8 (8 MiB/core
instead of 32 MiB).  The host maps spikes back with (y == 1) -> f32,
immune to whether the device's f32->u8 conversion saturates or wraps.

Per-core layout is chunk-major/t-minor: x[p, (c*T + t)*CH + n], so every
strip either engine touches is unit-stride.

Op placement (measured costs: DVE stt 1.21us, Act 1.15us per [128,1024]
strip): the serial recurrence A -> C -> A stays entirely on the DVE --
    A: m = (r mult 0.5) add x_t        scalar_tensor_tensor
    C: r = (m is_lt 1) mult m          scalar_tensor_tensor
with chunks interleaved t-outer/chunk-inner inside groups of 3, so the
in-order DVE stream always has another chunk's op between a chunk's C
and its next A: the chain never stalls (measured: 2us of DVE gaps over
the whole run).  Only the terminal spike op
    B: y_t = Sign(m - 1) -> uint8      activation
hangs off-chain on the Act engine.  Alternatives measured and rejected:
Pool stt is unsupported, Pool tensor_scalar runs at 16.5us, Pool
tensor_tensor (3.2us) stalls the chain when the reset is offloaded
(v5b: +53us), and PE identity-matmul updates need an on-chain
PSUM->SBUF copy (v3: every engine ~50% latency-stalled).

DMA: input strips ride the qSP hardware DGE queue only -- the SP
sequencer runs no compute, so strips always stream ahead of the DVE
(101us supply vs 136us demand); routing any strips via qAct stalled the
last group by 17us (Act issues them only after the prior group's B ops).
Spike strips are written back per-timestep on the software qPool queue
the moment each B completes, so the drain tail after the last spike is a
single 1 KiB/partition transfer.
"""

import numpy as np

import concourse.bass as bass
import concourse.tile as tile
from concourse import bacc, mybir
from concourse.bass_utils import run_bass_kernel_spmd

P = 128           # SBUF partitions
T = 8             # timesteps (innermost axis of the original input)
NPB = 8192        # neurons per partition per core: 8*128*32*32 / 128
CH = 2048         # neurons per chunk (per partition)
NCH = NPB // CH   # 8 chunks
GROUPS = [[0, 1], [2, 3]]

THRESH = 1.0
DECAY = 0.5
F32 = mybir.dt.float32
U8 = mybir.dt.uint8
N_CORES = 8

Alu = mybir.AluOpType
Act = mybir.ActivationFunctionType


def _build() -> bass.Bass:
    nc = bacc.Bacc("TRN2", target_bir_lowering=False, debug=False)
    x = nc.dram_tensor("x", [P, NCH * T * CH], F32, kind="ExternalInput").ap()
    y = nc.dram_tensor("y", [P, NCH * T * CH], U8, kind="ExternalOutput").ap()

    with tile.TileContext(nc) as tc:
        with (
            tc.tile_pool(name="consts", bufs=1) as cpool,
            tc.tile_pool(name="xs", bufs=16) as xpool,
            tc.tile_pool(name="ys", bufs=8) as ypool,
            tc.tile_pool(name="ms", bufs=3) as mpool,
            tc.tile_pool(name="rs", bufs=3) as rpool,
        ):
            neg_thresh = cpool.tile([P, 1], F32, tag="negth", name="neg_thresh")
            nc.gpsimd.memset(neg_thresh[:], -float(THRESH))

            def spike(c, t, m_ap):
                # B: u8 spike strip, written straight out on qPool.
                ys = ypool.tile([P, CH], U8, tag="y", name=f"y{c}_{t}")
                nc.scalar.activation(ys[:], m_ap, Act.Sign, bias=neg_thresh[:])
                nc.gpsimd.dma_start(
                    y[:, (c * T + t) * CH : (c * T + t + 1) * CH], ys[:]
                )

            for chunks in GROUPS:
                xs = {c: [None] * T for c in chunks}
                for t in range(T):
                    for c in chunks:
                        st = xpool.tile([P, CH], F32, tag="x", name=f"x{c}_{t}")
                        nc.sync.dma_start(
                            st[:], x[:, (c * T + t) * CH : (c * T + t + 1) * CH]
                        )
                        xs[c][t] = st

                r = {}
                for c in chunks:
                    r[c] = rpool.tile([P, CH], F32, tag="r", name=f"r{c}")

                # t = 0: mem0 = 0 so m == x_0 (read the strip directly).
                for c in chunks:
                    spike(c, 0, xs[c][0][:])
                for c in chunks:
                    nc.vector.scalar_tensor_tensor(
                        r[c][:], xs[c][0][:], THRESH, xs[c][0][:],
                        Alu.is_lt, Alu.mult,
                    )

                for t in range(1, T):
                    m = {}
                    for c in chunks:
                        ms = mpool.tile([P, CH], F32, tag="m", name=f"m{c}_{t}")
                        nc.vector.scalar_tensor_tensor(
                            ms[:], r[c][:], DECAY, xs[c][t][:],
                            Alu.mult, Alu.add,
                        )
                        m[c] = ms
                    for c in chunks:
                        spike(c, t, m[c][:])
                    if t < T - 1:
                        for c in chunks:
                            nc.vector.scalar_tensor_tensor(
                                r[c][:], m[c][:], THRESH, m[c][:],
                                Alu.is_lt, Alu.mult,
                            )
    nc.compile()
    return nc


_NC_CACHE: bass.Bass | None = None


def _get_nc() -> bass.Bass:
    global _NC_CACHE
    if _NC_CACHE is None:
        _NC_CACHE = _build()
    return _NC_CACHE


def _run(X: np.ndarray, **spmd_kwargs):
    assert X.shape == (64, 128, 32, 32, 8), X.shape
    X = np.ascontiguousarray(X, dtype=np.float32)
    per_core = 64 // N_CORES
    # [core, p, nch, ch, t] -> chunk-major t-minor [core, p, nch, t, ch]
    Xt = np.ascontiguousarray(
        X.reshape(N_CORES, P, NCH, CH, T).transpose(0, 1, 2, 4, 3)
    )
    in_maps = [{"x": Xt[i].reshape(P, NCH * T * CH)} for i in range(N_CORES)]
    res = run_bass_kernel_spmd(
        _get_nc(), in_maps, core_ids=list(range(N_CORES)), **spmd_kwargs
    )
    out = np.empty_like(X)
    for i, rr in enumerate(res.results):
        s = rr["y"].reshape(P, NCH, T, CH).transpose(0, 1, 3, 2)
        out[i * per_core : (i + 1) * per_core] = (
            (s == 1).astype(np.float32).reshape(per_core, 128, 32, 32, 8)
        )
    return out, res


def kernel(X: np.ndarray) -> np.ndarray:
    out, _ = _run(X)
    return out
